# revision 14
# baseline (speedup 1.0000x reference)
"""MultiHeadSelfAttention2D Trainium2 kernel (8-core SPMD, full I/O).

Problem: B=4, C_IN=C_OUT=256, HEADS=8, H=W=48 (m = 2304), fp32.
  vh, zh, qh = per-head 1x1-conv projections of x; rh = fixed 2D sin/cos PE.
  scores = vh^T zh + vh^T rh  (per b,h); attn = softmax(scores/sqrt(dh), axis=n)
  out = attn @ qh^T  -> (b, c_out, h, w)

Sharding: core = 2*b + head_half. Each core handles one batch image and 4
heads (=128 output channels). No cross-core communication.

Per-core pipeline (bf16 matmul path, fp32 psum accumulation; layouts chosen
so no on-chip transposes are needed):
  - vh2/kz2 [64, 2*2304] bf16: head h at row-block 32*(h%2), col-block h//2.
    Two row blocks let two scores matmuls run concurrently on disjoint PE
    row-groups while writing different psum banks (same-bank concurrent
    sub-array writes are fatal on TRN2).
  - qhT[n, ch] bf16, nb-major [128, 18*128]
  - for each 256-wide m-chunk, accumulate over 18 n-blocks in one psum bank:
      scoresT[n,m] 4 matmuls (2-way row-concurrent) -> psc [128, 1024]
      -> one ACT Exp (psum->sbuf bf16; ACT is the bottleneck engine)
      -> PV col-tiled 4-head matmuls; row-sums via ones-matmul every OTHER
         n-block on a DVE-precomputed et(j)+et(j+1) pair (halves PE's
         sums matmuls; PE runs HAM-throttled at 1.2 GHz on this workload)
  - DVE reciprocal + multiply for the softmax normalization, DMA out
"""

import numpy as np
from contextlib import ExitStack

import concourse.bass as bass
import concourse.bacc as bacc
import concourse.tile as tile
from concourse import mybir
from concourse.bass_utils import run_bass_kernel_spmd

F32 = mybir.dt.float32
BF16 = mybir.dt.bfloat16

B, C_IN, C_OUT, HEADS, H, W = 4, 256, 256, 8, 48, 48
M = H * W  # 2304
DH = C_OUT // HEADS  # 32
HPC = 4  # heads per core
CH = HPC * DH  # 128 channels per core
NB = M // 128  # 18 n-blocks of 128
MCH = 256  # m-chunk width
NMCH = M // MCH  # 9
INV_SCALE = float(1.0 / np.sqrt(np.float32(DH)))  # softmax temperature

# ACT (exact exp) and DVE (Schraudolph int16 bit-trick exp) split the
# softmax exponentials: the scalar engine was 76% busy doing all of them.
# DVE takes these n-blocks; bf16 bits = int16(psc*A + B) approximates
# exp(psc*INV_SCALE) to +-3% (softmax here is diffuse; error is harmless).
DVE_EXP_J = frozenset((2, 5, 9, 12, 15))
SCHRAUD_A = float(128.0 / np.log(2.0) * INV_SCALE)
# 127*128 (bf16 bias) - 5.51 (minimax centering) + 0.5 (trunc->round)
SCHRAUD_B = 16251.0

PROJ_CHUNKS = [(0, 512), (512, 512), (1024, 512), (1536, 512), (2048, 256)]

# blob column layout (per 128-partition row): x then the three weights
X_OFF = 0            # x  [128, 2, 2304]
WV_OFF = 2 * M       # 3x [128, 2, 128]
WZ_OFF = WV_OFF + 2 * CH
WQ_OFF = WZ_OFF + 2 * CH
BLOB_COLS = WQ_OFF + 2 * CH
# rh ships separately as [64, 2*2304] (pair layout, rows 0-63 only)

# scores psum / exp-tile column block per head: two-way row concurrency,
# h0/h2 share psum bank 0 (row block 0), h1/h3 share bank 1 (row block 32)
ECOL = [0, 2, 1, 3]  # head -> 256-col block in psc/et


def _kernel_body(ctx: ExitStack, tc: tile.TileContext, blob_d, rh_d, out_d):
    nc = tc.nc

    consts = ctx.enter_context(tc.tile_pool(name="consts", bufs=1))
    expp = ctx.enter_context(tc.tile_pool(name="expp", bufs=6))
    sump = ctx.enter_context(tc.tile_pool(name="sump", bufs=3))
    outp = ctx.enter_context(tc.tile_pool(name="outp", bufs=9))
    psum_s = ctx.enter_context(tc.tile_pool(name="psum_s", bufs=3, space="PSUM"))
    psum_o = ctx.enter_context(tc.tile_pool(name="psum_o", bufs=2, space="PSUM"))

    # ---- persistent SBUF tensors ----
    blob_sb = consts.tile([128, BLOB_COLS], F32)
    rh_sb = consts.tile([64, 2 * M], F32)
    x_bf = consts.tile([128, 2, M], BF16)
    wv_bf = consts.tile([128, 2, CH], BF16)
    wz_bf = consts.tile([128, 2, CH], BF16)
    wq_bf = consts.tile([128, 2, CH], BF16)
    vh2 = consts.tile([64, 2 * M], BF16)  # [32*(h%2)+c, (h//2)*M + m]
    kz2 = consts.tile([64, 2 * M], BF16)
    qt_sb = consts.tile([128, M], BF16)   # [n, ch] nb-major blocks
    ones_sb = consts.tile([128, DH], BF16)
    zeros_g = consts.tile([128, DH], BF16)  # ghost-matmul weights (all 0)

    # weights land first (small), then x/rh interleaved per chunk with both
    # contraction halves (k=0,1) adjacent, so projection chunk c can start as
    # soon as its ~0.77 MB slice arrives instead of after most of the input.
    nc.sync.dma_start(
        out=blob_sb[:, WV_OFF:BLOB_COLS],
        in_=blob_d.ap()[:, WV_OFF:BLOB_COLS],
    )
    for off, wd in PROJ_CHUNKS:
        for k in range(2):
            nc.sync.dma_start(
                out=blob_sb[:, k * M + off: k * M + off + wd],
                in_=blob_d.ap()[:, k * M + off: k * M + off + wd],
            )
        for pair in range(2):
            nc.sync.dma_start(
                out=rh_sb[:, pair * M + off: pair * M + off + wd],
                in_=rh_d.ap()[:, pair * M + off: pair * M + off + wd],
            )
    nc.vector.memset(ones_sb, 1.0)
    nc.vector.memset(zeros_g, 0.0)

    # bf16 working copies (DVE converts)
    nc.vector.tensor_copy(out=wv_bf, in_=blob_sb[:, WV_OFF:WV_OFF + 2 * CH])
    nc.vector.tensor_copy(out=wz_bf, in_=blob_sb[:, WZ_OFF:WZ_OFF + 2 * CH])
    nc.vector.tensor_copy(out=wq_bf, in_=blob_sb[:, WQ_OFF:WQ_OFF + 2 * CH])

    # zero bias for Exp, produced on ACT so the exp's bias dep is a free
    # same-engine edge rather than an extra cross-engine sync wait
    zero_bias = consts.tile([128, 1], F32)
    nc.scalar.mul(out=zero_bias, in_=blob_sb[:, WV_OFF:WV_OFF + 1], mul=0.0)

    # ---- projections (chunk-major; emission interleaved with attention) ----
    # vh2/kz2: head h -> psum rows 32*(h%2); vh in bank 0, kz in bank 1
    def emit_proj_chunk(off, wd):
        for k in range(2):
            nc.vector.tensor_copy(
                out=x_bf[:, k, off:off + wd],
                in_=blob_sb[:, k * M + off: k * M + off + wd],
            )
        for pair in range(2):
            ps = psum_s.tile([128, 4 * MCH], F32, tag="psc", name="ps")
            for hh in range(2):  # head = 2*pair + hh
                h = 2 * pair + hh
                for k in range(2):
                    nc.tensor.matmul(
                        ps[32 * hh:32 * hh + 32, :wd],
                        lhsT=wv_bf[:, k, 32 * h:32 * h + 32],
                        rhs=x_bf[:, k, off:off + wd],
                        start=(k == 0),
                        stop=(k == 1),
                        tile_position=(0, 32 * hh),
                        skip_group_check=True,
                    )
                    nc.tensor.matmul(
                        ps[32 * hh:32 * hh + 32, 512:512 + wd],
                        lhsT=wz_bf[:, k, 32 * h:32 * h + 32],
                        rhs=x_bf[:, k, off:off + wd],
                        start=(k == 0),
                        stop=(k == 1),
                        tile_position=(0, 32 * hh),
                        skip_group_check=True,
                    )
            nc.vector.tensor_copy(
                out=vh2[:, pair * M + off: pair * M + off + wd], in_=ps[0:64, :wd]
            )
            nc.vector.tensor_add(
                out=kz2[:, pair * M + off: pair * M + off + wd],
                in0=ps[0:64, 512:512 + wd],
                in1=rh_sb[:, pair * M + off: pair * M + off + wd],
            )
        # qhT for the n-blocks covered by this chunk:
        # out[n, ch] = sum_cin x[cin, n] * wT[cin, ch]
        for nb in range(off // 128, (off + wd) // 128):
            ps = psum_s.tile([128, 4 * MCH], F32, tag="psc", name="ps")
            for k in range(2):
                nc.tensor.matmul(
                    ps[:, :CH],
                    lhsT=x_bf[:, k, nb * 128:(nb + 1) * 128],
                    rhs=wq_bf[:, k, :],
                    start=(k == 0),
                    stop=(k == 1),
                )
            nc.vector.tensor_copy(out=qt_sb[:, nb * 128:(nb + 1) * 128], in_=ps[:, :CH])

    # ---- attention ----
    def emit_scores(psc, j, m0):
        # h0 (rows 0-31 -> bank0) || h1 (rows 32-63 -> bank1) concurrent;
        # h2/h3 reuse the same PE rows so they serialize behind h0/h1.
        for h in (0, 1, 2, 3):
            rb = 32 * (h % 2)
            cb = (h // 2) * M
            nc.tensor.matmul(
                psc[:, ECOL[h] * MCH:(ECOL[h] + 1) * MCH],
                lhsT=kz2[rb:rb + 32, cb + j * 128: cb + (j + 1) * 128],
                rhs=vh2[rb:rb + 32, cb + m0: cb + m0 + MCH],
                start=True,
                stop=True,
                tile_position=(rb, 0),
                skip_group_check=True,
            )

    def emit_pv(po, j, et):
        # PV: out_unnorm[32h+d, m] += sum_n qhT[n, 32h+d] * expT_h[n, m]
        for h in range(HPC):
            nc.tensor.matmul(
                po[32 * h:32 * h + 32, 0:MCH],
                lhsT=qt_sb[:, j * 128 + 32 * h: j * 128 + 32 * h + 32],
                rhs=et[:, ECOL[h] * MCH:(ECOL[h] + 1) * MCH],
                start=(j == 0),
                stop=False,
                tile_position=(0, 32 * h),
                skip_group_check=True,
            )

    def emit_sums(po, etp, last):
        # row-sums of an et(j)+et(j+1) pair, replicated over each head slot
        for h in range(HPC):
            nc.tensor.matmul(
                po[32 * h:32 * h + 32, MCH:2 * MCH],
                lhsT=ones_sb,
                rhs=etp[:, ECOL[h] * MCH:(ECOL[h] + 1) * MCH],
                start=False,
                stop=last,
                tile_position=(0, 32 * h),
                skip_group_check=True,
            )

    def emit_pair_sums(po, ets, pj, last):
        etp = sump.tile([128, 4 * MCH], BF16, tag="etp")
        nc.vector.tensor_add(out=etp, in0=ets[pj - 1], in1=ets[pj])
        emit_sums(po, etp, last)

    # Flat software-pipelined loop over g = mc*NB + j. Scores run SLAG steps
    # ahead of exp, which runs one ahead of PV/sums, so the PE queue head
    # never waits on an exp result (head-of-line blocking kept PE idle in
    # bursts, which also pins the HAM clock-gate at 1.2 GHz).
    SLAG = 2  # scores stream leads the exp stream by this many steps
    G = NMCH * NB
    pscs, ets, pos = {}, {}, {}

    def emit_scores_g(g):
        if not (0 <= g < G):
            return
        mc, j = divmod(g, NB)
        pscs[g] = psum_s.tile([128, 4 * MCH], F32, tag="psc", name="psc")
        emit_scores(pscs[g], j, mc * MCH)

    def emit_exp_g(g):
        if not (0 <= g < G):
            return
        _, j = divmod(g, NB)
        psc = pscs[g]
        et = expp.tile([128, 4 * MCH], BF16, tag="et")
        if j in DVE_EXP_J:
            nc.vector.tensor_scalar(
                out=et.bitcast(mybir.dt.int16), in0=psc,
                scalar1=SCHRAUD_A, scalar2=SCHRAUD_B,
                op0=mybir.AluOpType.mult, op1=mybir.AluOpType.add,
            )
        else:
            nc.scalar.activation(
                out=et, in_=psc, func=mybir.ActivationFunctionType.Exp,
                bias=zero_bias, scale=INV_SCALE,
            )
        ets[g] = et

    def emit_pv_g(g):
        if not (0 <= g < G):
            return
        mc, j = divmod(g, NB)
        if j == 0:
            pos[mc] = psum_o.tile([128, 2 * MCH], F32, tag="po", name="po")
        po = pos[mc]
        emit_pv(po, j, ets[g])
        if j % 2 == 1:
            etp = sump.tile([128, 4 * MCH], BF16, tag="etp")
            nc.vector.tensor_add(out=etp, in0=ets.pop(g - 1), in1=ets[g])
            emit_sums(po, etp, last=(j == NB - 1))
        if j % 2 == 1 or j == NB - 1:
            ets.pop(g)
        if j == NB - 1:
            po = pos.pop(mc)
            rc = outp.tile([128, MCH], F32, tag="recip")
            nc.vector.reciprocal_approx_fast(out=rc, in_=po[:, MCH:2 * MCH])
            of = outp.tile([128, MCH], F32, tag="outf")
            nc.vector.tensor_mul(out=of, in0=po[:, 0:MCH], in1=rc)
            m0 = mc * MCH
            nc.sync.dma_start(out=out_d.ap()[:, m0:m0 + MCH], in_=of)

    def emit_ghost(g, idx):
        # Zero-weight matmul on PE rows 64-127 (idle during the scores phase,
        # which only streams rows 0-63) into the already-exp'd psc(g) buffer.
        # Results are x*0 into a dead buffer that scores(g+3) start=True
        # clears; the only purpose is stream activity so the HAM clock gate
        # keeps the PE at 2.4 GHz instead of re-throttling to 1.2.
        if not (NB <= g < G) or g not in pscs:
            return
        mc = g // NB
        m0 = (mc % NMCH) * MCH
        nc.tensor.matmul(
            pscs[g][32 * idx:32 * idx + 32, 0:MCH],
            lhsT=zeros_g[64:128, :],
            rhs=x_bf[64:128, 0, m0:m0 + MCH],
            start=True,
            stop=True,
            tile_position=(64, 32 * idx),
            skip_group_check=True,
        )

    # Interleave projection-chunk emission with the attention pipeline:
    # scores for n-block j only need x/kz chunk j//4, so later chunks project
    # while the first n-blocks' exps already stream on ACT/DVE.
    next_chunk = 0

    def ensure_chunks_for_scores(gs):
        nonlocal next_chunk
        if not (0 <= gs < G):
            need = len(PROJ_CHUNKS)  # pipeline done; flush any stragglers
        else:
            j = gs % NB
            need = min(j // 4, len(PROJ_CHUNKS) - 1) + 1 if gs < NB else len(PROJ_CHUNKS)
        while next_chunk < need:
            emit_proj_chunk(*PROJ_CHUNKS[next_chunk])
            next_chunk += 1

    for g in range(-SLAG, G):
        ensure_chunks_for_scores(g + SLAG)
        emit_ghost(g, 0)
        emit_scores_g(g + SLAG)
        emit_ghost(g, 1)
        emit_exp_g(g + 1)
        emit_pv_g(g)
        pscs.pop(g, None)

def build_module() -> bass.Bass:
    nc = bacc.Bacc("TRN2", target_bir_lowering=False)
    blob_d = nc.declare_dram_parameter("blob", [128, BLOB_COLS], F32, isOutput=False)
    rh_d = nc.declare_dram_parameter("rh2", [64, 2 * M], F32, isOutput=False)
    out_d = nc.declare_dram_parameter("out", [CH, M], F32, isOutput=True)
    with tile.TileContext(nc) as tc, ExitStack() as ctx:
        _kernel_body(ctx, tc, blob_d, rh_d, out_d)
    nc.compile()
    return nc


def pos_encoding_2d(c, h, w):
    """numpy port of the reference's fixed 2D sinusoidal PE -> (c, h*w)."""
    ch = c // 2
    div = np.float32(10000.0) ** (np.arange(0, ch, 2, dtype=np.float32) / np.float32(ch))
    py = np.arange(h, dtype=np.float32)[None, :] / div[:, None]
    px = np.arange(w, dtype=np.float32)[None, :] / div[:, None]
    pe_y = np.stack([np.sin(py), np.cos(py)], axis=1).reshape(ch, h).astype(np.float32)
    pe_x = np.stack([np.sin(px), np.cos(px)], axis=1).reshape(ch, w).astype(np.float32)
    pe = np.concatenate(
        [
            np.broadcast_to(pe_y[:, :, None], (ch, h, w)),
            np.broadcast_to(pe_x[:, None, :], (ch, h, w)),
        ],
        axis=0,
    )
    return np.ascontiguousarray(pe.reshape(c, h * w), dtype=np.float32)


_CACHE = {}


def _get_nc() -> bass.Bass:
    if "nc" not in _CACHE:
        _CACHE["nc"] = build_module()
    return _CACHE["nc"]


def make_in_maps(x, w_v, w_z, w_q):
    rh_full = pos_encoding_2d(C_OUT, H, W)
    x = np.asarray(x, dtype=np.float32)
    w_v = np.asarray(w_v, dtype=np.float32)
    w_z = np.asarray(w_z, dtype=np.float32)
    w_q = np.asarray(w_q, dtype=np.float32)
    in_maps = []
    for core in range(8):
        b, hh = core // 2, core % 2
        c0 = CH * hh
        blob = np.zeros((128, BLOB_COLS), np.float32)
        # x: blob[p, k*M + j] = x[b, k*128 + p, j]
        xx = x[b].reshape(2, 128, M)
        blob[:, X_OFF:X_OFF + 2 * M] = xx.transpose(1, 0, 2).reshape(128, 2 * M)
        # weights: blob[p, base + k*CH + c] = w[c0 + c, k*128 + p]
        for base, wm in ((WV_OFF, w_v), (WZ_OFF, w_z), (WQ_OFF, w_q)):
            wt = wm[c0:c0 + CH, :].T.reshape(2, 128, CH)  # [k, p, c]
            blob[:, base:base + 2 * CH] = wt.transpose(1, 0, 2).reshape(128, 2 * CH)
        # rh2: [32*(h%2)+c, (h//2)*M + m] = rh[c0 + 32h + c, m]  (rows 0-63)
        rh2 = np.zeros((64, 2 * M), np.float32)
        rh_c = rh_full[c0:c0 + CH, :].reshape(4, DH, M)  # [h, c, m]
        for h in range(4):
            r0 = DH * (h % 2)
            cb = (h // 2) * M
            rh2[r0:r0 + DH, cb:cb + M] = rh_c[h]
        in_maps.append({"blob": blob, "rh2": rh2})
    return in_maps


def assemble_output(results):
    out = np.empty((B, C_OUT, H, W), np.float32)
    for core in range(8):
        b, hh = core // 2, core % 2
        out[b, CH * hh:CH * hh + CH] = results[core]["out"].reshape(CH, H, W)
    return out


def kernel(x, w_v, w_z, w_q, _trace=False):
    nc = _get_nc()
    in_maps = make_in_maps(x, w_v, w_z, w_q)
    res = run_bass_kernel_spmd(nc, in_maps, core_ids=list(range(8)), trace=_trace)
    out = assemble_output(res.results)
    if _trace:
        kernel.last_results = res
    return out



# revision 15
# speedup vs baseline: 1.4456x; 1.4456x over previous
"""MultiHeadSelfAttention2D Trainium2 kernel (8-core SPMD, full I/O).

Problem: B=4, C_IN=C_OUT=256, HEADS=8, H=W=48 (m = 2304), fp32.
  vh, zh, qh = per-head 1x1-conv projections of x; rh = fixed 2D sin/cos PE.
  scores = vh^T zh + vh^T rh  (per b,h); attn = softmax(scores/sqrt(dh), axis=n)
  out = attn @ qh^T  -> (b, c_out, h, w)

Sharding: core = 2*b + head_half. Each core handles one batch image and 4
heads (=128 output channels). No cross-core communication.

Per-core pipeline (bf16 matmul path, fp32 psum accumulation; layouts chosen
so no on-chip transposes are needed):
  - vh2/kz2 [64, 2*2304] bf16: head h at row-block 32*(h%2), col-block h//2.
    Two row blocks let two scores matmuls run concurrently on disjoint PE
    row-groups while writing different psum banks (same-bank concurrent
    sub-array writes are fatal on TRN2).
  - qhT[n, ch] bf16, nb-major [128, 18*128]
  - for each 256-wide m-chunk, accumulate over 18 n-blocks in one psum bank:
      scoresT[n,m] 4 matmuls (2-way row-concurrent) -> psc [128, 1024]
      -> one ACT Exp (psum->sbuf bf16; ACT is the bottleneck engine)
      -> PV col-tiled 4-head matmuls; row-sums via ones-matmul every OTHER
         n-block on a DVE-precomputed et(j)+et(j+1) pair (halves PE's
         sums matmuls; PE runs HAM-throttled at 1.2 GHz on this workload)
  - DVE reciprocal + multiply for the softmax normalization, DMA out
"""

import numpy as np
from contextlib import ExitStack

import concourse.bass as bass
import concourse.bacc as bacc
import concourse.tile as tile
from concourse import mybir
from concourse.bass_utils import run_bass_kernel_spmd

F32 = mybir.dt.float32
BF16 = mybir.dt.bfloat16

B, C_IN, C_OUT, HEADS, H, W = 4, 256, 256, 8, 48, 48
M = H * W  # 2304
DH = C_OUT // HEADS  # 32
HPC = 4  # heads per core
CH = HPC * DH  # 128 channels per core
NB = M // 128  # 18 n-blocks of 128
MCH = 256  # m-chunk width
NMCH = M // MCH  # 9
INV_SCALE = float(1.0 / np.sqrt(np.float32(DH)))  # softmax temperature

# ACT (exact exp) and DVE (Schraudolph int16 bit-trick exp) split the
# softmax exponentials: the scalar engine was 76% busy doing all of them.
# DVE takes these n-blocks; bf16 bits = int16(psc*A + B) approximates
# exp(psc*INV_SCALE) to +-3% (softmax here is diffuse; error is harmless).
DVE_EXP_J = frozenset((2, 5, 9, 12, 15))
SCHRAUD_A = float(128.0 / np.log(2.0) * INV_SCALE)
# 127*128 (bf16 bias) - 5.51 (minimax centering) + 0.5 (trunc->round)
SCHRAUD_B = 16251.0

PROJ_CHUNKS = [(0, 512), (512, 512), (1024, 512), (1536, 512), (2048, 256)]

# blob column layout (per 128-partition row): x then the three weights
X_OFF = 0            # x  [128, 2, 2304]
WV_OFF = 2 * M       # 3x [128, 2, 128]
WZ_OFF = WV_OFF + 2 * CH
WQ_OFF = WZ_OFF + 2 * CH
BLOB_COLS = WQ_OFF + 2 * CH
# rh ships separately as [64, 2*2304] (pair layout, rows 0-63 only)

# scores psum / exp-tile column block per head: two-way row concurrency,
# h0/h2 share psum bank 0 (row block 0), h1/h3 share bank 1 (row block 32)
ECOL = [0, 2, 1, 3]  # head -> 256-col block in psc/et


def _kernel_body(ctx: ExitStack, tc: tile.TileContext, blob_d, rh_d, out_d):
    nc = tc.nc

    consts = ctx.enter_context(tc.tile_pool(name="consts", bufs=1))
    expp = ctx.enter_context(tc.tile_pool(name="expp", bufs=6))
    sump = ctx.enter_context(tc.tile_pool(name="sump", bufs=3))
    outp = ctx.enter_context(tc.tile_pool(name="outp", bufs=9))
    psum_s = ctx.enter_context(tc.tile_pool(name="psum_s", bufs=3, space="PSUM"))
    psum_o = ctx.enter_context(tc.tile_pool(name="psum_o", bufs=2, space="PSUM"))

    # ---- persistent SBUF tensors ----
    blob_sb = consts.tile([128, BLOB_COLS], F32)
    rh_sb = consts.tile([64, 2 * M], F32)
    x_bf = consts.tile([128, 2, M], BF16)
    wv_bf = consts.tile([128, 2, CH], BF16)
    wz_bf = consts.tile([128, 2, CH], BF16)
    wq_bf = consts.tile([128, 2, CH], BF16)
    vh2 = consts.tile([64, 2 * M], BF16)  # [32*(h%2)+c, (h//2)*M + m]
    kz2 = consts.tile([64, 2 * M], BF16)
    qt_sb = consts.tile([128, M], BF16)   # [n, ch] nb-major blocks
    ones_sb = consts.tile([128, DH], BF16)
    zeros_g = consts.tile([128, DH], BF16)  # ghost-matmul weights (all 0)

    # weights land first (small), then x/rh interleaved per chunk with both
    # contraction halves (k=0,1) adjacent, so projection chunk c can start as
    # soon as its ~0.77 MB slice arrives instead of after most of the input.
    nc.sync.dma_start(
        out=blob_sb[:, WV_OFF:BLOB_COLS],
        in_=blob_d.ap()[:, WV_OFF:BLOB_COLS],
    )
    for off, wd in PROJ_CHUNKS:
        for k in range(2):
            nc.sync.dma_start(
                out=blob_sb[:, k * M + off: k * M + off + wd],
                in_=blob_d.ap()[:, k * M + off: k * M + off + wd],
            )
        for pair in range(2):
            nc.sync.dma_start(
                out=rh_sb[:, pair * M + off: pair * M + off + wd],
                in_=rh_d.ap()[:, pair * M + off: pair * M + off + wd],
            )
    nc.vector.memset(ones_sb, 1.0)
    nc.vector.memset(zeros_g, 0.0)

    # bf16 working copies (DVE converts)
    nc.vector.tensor_copy(out=wv_bf, in_=blob_sb[:, WV_OFF:WV_OFF + 2 * CH])
    nc.vector.tensor_copy(out=wz_bf, in_=blob_sb[:, WZ_OFF:WZ_OFF + 2 * CH])
    nc.vector.tensor_copy(out=wq_bf, in_=blob_sb[:, WQ_OFF:WQ_OFF + 2 * CH])

    # zero bias for Exp, produced on ACT so the exp's bias dep is a free
    # same-engine edge rather than an extra cross-engine sync wait
    zero_bias = consts.tile([128, 1], F32)
    nc.scalar.mul(out=zero_bias, in_=blob_sb[:, WV_OFF:WV_OFF + 1], mul=0.0)

    # ---- projections (chunk-major; emission interleaved with attention) ----
    # vh2/kz2: head h -> psum rows 32*(h%2); vh in bank 0, kz in bank 1
    def emit_proj_chunk(off, wd):
        for k in range(2):
            nc.vector.tensor_copy(
                out=x_bf[:, k, off:off + wd],
                in_=blob_sb[:, k * M + off: k * M + off + wd],
            )
        for pair in range(2):
            ps = psum_s.tile([128, 4 * MCH], F32, tag="psc", name="ps")
            for hh in range(2):  # head = 2*pair + hh
                h = 2 * pair + hh
                for k in range(2):
                    nc.tensor.matmul(
                        ps[32 * hh:32 * hh + 32, :wd],
                        lhsT=wv_bf[:, k, 32 * h:32 * h + 32],
                        rhs=x_bf[:, k, off:off + wd],
                        start=(k == 0),
                        stop=(k == 1),
                        tile_position=(0, 32 * hh),
                        skip_group_check=True,
                    )
                    nc.tensor.matmul(
                        ps[32 * hh:32 * hh + 32, 512:512 + wd],
                        lhsT=wz_bf[:, k, 32 * h:32 * h + 32],
                        rhs=x_bf[:, k, off:off + wd],
                        start=(k == 0),
                        stop=(k == 1),
                        tile_position=(0, 32 * hh),
                        skip_group_check=True,
                    )
            nc.vector.tensor_copy(
                out=vh2[:, pair * M + off: pair * M + off + wd], in_=ps[0:64, :wd]
            )
            nc.vector.tensor_add(
                out=kz2[:, pair * M + off: pair * M + off + wd],
                in0=ps[0:64, 512:512 + wd],
                in1=rh_sb[:, pair * M + off: pair * M + off + wd],
            )
        # qhT for the n-blocks covered by this chunk:
        # out[n, ch] = sum_cin x[cin, n] * wT[cin, ch]
        for nb in range(off // 128, (off + wd) // 128):
            ps = psum_s.tile([128, 4 * MCH], F32, tag="psc", name="ps")
            for k in range(2):
                nc.tensor.matmul(
                    ps[:, :CH],
                    lhsT=x_bf[:, k, nb * 128:(nb + 1) * 128],
                    rhs=wq_bf[:, k, :],
                    start=(k == 0),
                    stop=(k == 1),
                )
            nc.vector.tensor_copy(out=qt_sb[:, nb * 128:(nb + 1) * 128], in_=ps[:, :CH])

    # ---- attention ----
    def emit_scores(psc, j, m0):
        # h0 (rows 0-31 -> bank0) || h1 (rows 32-63 -> bank1) concurrent;
        # h2/h3 reuse the same PE rows so they serialize behind h0/h1.
        for h in (0, 1, 2, 3):
            rb = 32 * (h % 2)
            cb = (h // 2) * M
            nc.tensor.matmul(
                psc[:, ECOL[h] * MCH:(ECOL[h] + 1) * MCH],
                lhsT=kz2[rb:rb + 32, cb + j * 128: cb + (j + 1) * 128],
                rhs=vh2[rb:rb + 32, cb + m0: cb + m0 + MCH],
                start=True,
                stop=True,
                tile_position=(rb, 0),
                skip_group_check=True,
            )

    def emit_pv(po, j, et):
        # PV: out_unnorm[32h+d, m] += sum_n qhT[n, 32h+d] * expT_h[n, m]
        for h in range(HPC):
            nc.tensor.matmul(
                po[32 * h:32 * h + 32, 0:MCH],
                lhsT=qt_sb[:, j * 128 + 32 * h: j * 128 + 32 * h + 32],
                rhs=et[:, ECOL[h] * MCH:(ECOL[h] + 1) * MCH],
                start=(j == 0),
                stop=False,
                tile_position=(0, 32 * h),
                skip_group_check=True,
            )

    def emit_sums(po, etp, last):
        # row-sums of an et(j)+et(j+1) pair, replicated over each head slot
        for h in range(HPC):
            nc.tensor.matmul(
                po[32 * h:32 * h + 32, MCH:2 * MCH],
                lhsT=ones_sb,
                rhs=etp[:, ECOL[h] * MCH:(ECOL[h] + 1) * MCH],
                start=False,
                stop=last,
                tile_position=(0, 32 * h),
                skip_group_check=True,
            )

    def emit_pair_sums(po, ets, pj, last):
        etp = sump.tile([128, 4 * MCH], BF16, tag="etp")
        nc.vector.tensor_add(out=etp, in0=ets[pj - 1], in1=ets[pj])
        emit_sums(po, etp, last)

    # Flat software-pipelined loop over g = mc*NB + j. Scores run SLAG steps
    # ahead of exp, which runs one ahead of PV/sums, so the PE queue head
    # never waits on an exp result (head-of-line blocking kept PE idle in
    # bursts, which also pins the HAM clock-gate at 1.2 GHz).
    SLAG = 2  # scores stream leads the exp stream by this many steps
    G = NMCH * NB
    pscs, ets, pos = {}, {}, {}

    def emit_scores_g(g):
        if not (0 <= g < G):
            return
        mc, j = divmod(g, NB)
        pscs[g] = psum_s.tile([128, 4 * MCH], F32, tag="psc", name="psc")
        emit_scores(pscs[g], j, mc * MCH)

    def emit_exp_g(g):
        if not (0 <= g < G):
            return
        _, j = divmod(g, NB)
        psc = pscs[g]
        et = expp.tile([128, 4 * MCH], BF16, tag="et")
        if j in DVE_EXP_J:
            nc.vector.tensor_scalar(
                out=et.bitcast(mybir.dt.int16), in0=psc,
                scalar1=SCHRAUD_A, scalar2=SCHRAUD_B,
                op0=mybir.AluOpType.mult, op1=mybir.AluOpType.add,
            )
        else:
            nc.scalar.activation(
                out=et, in_=psc, func=mybir.ActivationFunctionType.Exp,
                bias=zero_bias, scale=INV_SCALE,
            )
        ets[g] = et

    def emit_pv_g(g):
        if not (0 <= g < G):
            return
        mc, j = divmod(g, NB)
        if j == 0:
            pos[mc] = psum_o.tile([128, 2 * MCH], F32, tag="po", name="po")
        po = pos[mc]
        emit_pv(po, j, ets[g])
        if j % 2 == 1:
            etp = sump.tile([128, 4 * MCH], BF16, tag="etp")
            nc.vector.tensor_add(out=etp, in0=ets.pop(g - 1), in1=ets[g])
            emit_sums(po, etp, last=(j == NB - 1))
        if j % 2 == 1 or j == NB - 1:
            ets.pop(g)
        if j == NB - 1:
            po = pos.pop(mc)
            rc = outp.tile([128, MCH], F32, tag="recip")
            nc.vector.reciprocal_approx_fast(out=rc, in_=po[:, MCH:2 * MCH])
            of = outp.tile([128, MCH], F32, tag="outf")
            nc.vector.tensor_mul(out=of, in0=po[:, 0:MCH], in1=rc)
            m0 = mc * MCH
            nc.sync.dma_start(out=out_d.ap()[:, m0:m0 + MCH], in_=of)

    def emit_ghost(g, idx):
        # Zero-weight matmul on PE rows 64-127 (idle during the scores phase,
        # which only streams rows 0-63) into the already-exp'd psc(g) buffer.
        # Results are x*0 into a dead buffer that scores(g+3) start=True
        # clears; the only purpose is stream activity so the HAM clock gate
        # keeps the PE at 2.4 GHz instead of re-throttling to 1.2.
        if not (NB <= g < G) or g not in pscs:
            return
        mc = g // NB
        m0 = (mc % NMCH) * MCH
        nc.tensor.matmul(
            pscs[g][32 * idx:32 * idx + 32, 0:MCH],
            lhsT=zeros_g[64:128, :],
            rhs=x_bf[64:128, 0, m0:m0 + MCH],
            start=True,
            stop=True,
            tile_position=(64, 32 * idx),
            skip_group_check=True,
        )

    # Interleave projection-chunk emission with the attention pipeline:
    # scores for n-block j only need x/kz chunk j//4, so later chunks project
    # while the first n-blocks' exps already stream on ACT/DVE.
    next_chunk = 0

    def ensure_chunks_for_scores(gs):
        nonlocal next_chunk
        if not (0 <= gs < G):
            need = len(PROJ_CHUNKS)  # pipeline done; flush any stragglers
        else:
            j = gs % NB
            need = min(j // 4, len(PROJ_CHUNKS) - 1) + 1 if gs < NB else len(PROJ_CHUNKS)
        while next_chunk < need:
            emit_proj_chunk(*PROJ_CHUNKS[next_chunk])
            next_chunk += 1

    for g in range(-SLAG, G):
        ensure_chunks_for_scores(g + SLAG)
        emit_scores_g(g + SLAG)
        emit_exp_g(g + 1)
        emit_pv_g(g)
        pscs.pop(g, None)

def build_module() -> bass.Bass:
    nc = bacc.Bacc("TRN2", target_bir_lowering=False)
    blob_d = nc.declare_dram_parameter("blob", [128, BLOB_COLS], F32, isOutput=False)
    rh_d = nc.declare_dram_parameter("rh2", [64, 2 * M], F32, isOutput=False)
    out_d = nc.declare_dram_parameter("out", [CH, M], F32, isOutput=True)
    with tile.TileContext(nc) as tc, ExitStack() as ctx:
        _kernel_body(ctx, tc, blob_d, rh_d, out_d)
    nc.compile()
    return nc


def pos_encoding_2d(c, h, w):
    """numpy port of the reference's fixed 2D sinusoidal PE -> (c, h*w)."""
    ch = c // 2
    div = np.float32(10000.0) ** (np.arange(0, ch, 2, dtype=np.float32) / np.float32(ch))
    py = np.arange(h, dtype=np.float32)[None, :] / div[:, None]
    px = np.arange(w, dtype=np.float32)[None, :] / div[:, None]
    pe_y = np.stack([np.sin(py), np.cos(py)], axis=1).reshape(ch, h).astype(np.float32)
    pe_x = np.stack([np.sin(px), np.cos(px)], axis=1).reshape(ch, w).astype(np.float32)
    pe = np.concatenate(
        [
            np.broadcast_to(pe_y[:, :, None], (ch, h, w)),
            np.broadcast_to(pe_x[:, None, :], (ch, h, w)),
        ],
        axis=0,
    )
    return np.ascontiguousarray(pe.reshape(c, h * w), dtype=np.float32)


_CACHE = {}


def _get_nc() -> bass.Bass:
    if "nc" not in _CACHE:
        _CACHE["nc"] = build_module()
    return _CACHE["nc"]


def make_in_maps(x, w_v, w_z, w_q):
    rh_full = pos_encoding_2d(C_OUT, H, W)
    x = np.asarray(x, dtype=np.float32)
    w_v = np.asarray(w_v, dtype=np.float32)
    w_z = np.asarray(w_z, dtype=np.float32)
    w_q = np.asarray(w_q, dtype=np.float32)
    in_maps = []
    for core in range(8):
        b, hh = core // 2, core % 2
        c0 = CH * hh
        blob = np.zeros((128, BLOB_COLS), np.float32)
        # x: blob[p, k*M + j] = x[b, k*128 + p, j]
        xx = x[b].reshape(2, 128, M)
        blob[:, X_OFF:X_OFF + 2 * M] = xx.transpose(1, 0, 2).reshape(128, 2 * M)
        # weights: blob[p, base + k*CH + c] = w[c0 + c, k*128 + p]
        for base, wm in ((WV_OFF, w_v), (WZ_OFF, w_z), (WQ_OFF, w_q)):
            wt = wm[c0:c0 + CH, :].T.reshape(2, 128, CH)  # [k, p, c]
            blob[:, base:base + 2 * CH] = wt.transpose(1, 0, 2).reshape(128, 2 * CH)
        # rh2: [32*(h%2)+c, (h//2)*M + m] = rh[c0 + 32h + c, m]  (rows 0-63)
        rh2 = np.zeros((64, 2 * M), np.float32)
        rh_c = rh_full[c0:c0 + CH, :].reshape(4, DH, M)  # [h, c, m]
        for h in range(4):
            r0 = DH * (h % 2)
            cb = (h // 2) * M
            rh2[r0:r0 + DH, cb:cb + M] = rh_c[h]
        in_maps.append({"blob": blob, "rh2": rh2})
    return in_maps


def assemble_output(results):
    out = np.empty((B, C_OUT, H, W), np.float32)
    for core in range(8):
        b, hh = core // 2, core % 2
        out[b, CH * hh:CH * hh + CH] = results[core]["out"].reshape(CH, H, W)
    return out


def kernel(x, w_v, w_z, w_q, _trace=False):
    nc = _get_nc()
    in_maps = make_in_maps(x, w_v, w_z, w_q)
    res = run_bass_kernel_spmd(nc, in_maps, core_ids=list(range(8)), trace=_trace)
    out = assemble_output(res.results)
    if _trace:
        kernel.last_results = res
    return out



# revision 17
# speedup vs baseline: 1.4541x; 1.0059x over previous
"""MultiHeadSelfAttention2D Trainium2 kernel (8-core SPMD, full I/O).

Problem: B=4, C_IN=C_OUT=256, HEADS=8, H=W=48 (m = 2304), fp32.
  vh, zh, qh = per-head 1x1-conv projections of x; rh = fixed 2D sin/cos PE.
  scores = vh^T zh + vh^T rh  (per b,h); attn = softmax(scores/sqrt(dh), axis=n)
  out = attn @ qh^T  -> (b, c_out, h, w)

Sharding: core = 2*b + head_half. Each core handles one batch image and 4
heads (=128 output channels). No cross-core communication.

Per-core pipeline (bf16 matmul path, fp32 psum accumulation; layouts chosen
so no on-chip transposes are needed):
  - vh2/kz2 [64, 2*2304] bf16: head h at row-block 32*(h%2), col-block h//2.
    Two row blocks let two scores matmuls run concurrently on disjoint PE
    row-groups while writing different psum banks (same-bank concurrent
    sub-array writes are fatal on TRN2).
  - qhT[n, ch] bf16, nb-major [128, 18*128]
  - for each 256-wide m-chunk, accumulate over 18 n-blocks in one psum bank:
      scoresT[n,m] 4 matmuls (2-way row-concurrent) -> psc [128, 1024]
      -> one ACT Exp (psum->sbuf bf16; ACT is the bottleneck engine)
      -> PV col-tiled 4-head matmuls; row-sums via ones-matmul every OTHER
         n-block on a DVE-precomputed et(j)+et(j+1) pair (halves PE's
         sums matmuls; PE runs HAM-throttled at 1.2 GHz on this workload)
  - DVE reciprocal + multiply for the softmax normalization, DMA out
"""

import numpy as np
from contextlib import ExitStack

import concourse.bass as bass
import concourse.bacc as bacc
import concourse.tile as tile
from concourse import mybir
from concourse.bass_utils import run_bass_kernel_spmd

F32 = mybir.dt.float32
BF16 = mybir.dt.bfloat16

B, C_IN, C_OUT, HEADS, H, W = 4, 256, 256, 8, 48, 48
M = H * W  # 2304
DH = C_OUT // HEADS  # 32
HPC = 4  # heads per core
CH = HPC * DH  # 128 channels per core
NB = M // 128  # 18 n-blocks of 128
MCH = 256  # m-chunk width
NMCH = M // MCH  # 9
INV_SCALE = float(1.0 / np.sqrt(np.float32(DH)))  # softmax temperature

# ACT (exact exp) and DVE (Schraudolph int16 bit-trick exp) split the
# softmax exponentials: the scalar engine was 76% busy doing all of them.
# DVE takes these n-blocks; bf16 bits = int16(psc*A + B) approximates
# exp(psc*INV_SCALE) to +-3% (softmax here is diffuse; error is harmless).
DVE_EXP_J = frozenset((2, 5, 9, 12, 15))
SCHRAUD_A = float(128.0 / np.log(2.0) * INV_SCALE)
# 127*128 (bf16 bias) - 5.51 (minimax centering) + 0.5 (trunc->round)
SCHRAUD_B = 16251.0

PROJ_CHUNKS = [(0, 512), (512, 512), (1024, 512), (1536, 512), (2048, 256)]

# blob column layout (per 128-partition row): x then the three weights
X_OFF = 0            # x  [128, 2, 2304]
WV_OFF = 2 * M       # 3x [128, 2, 128]
WZ_OFF = WV_OFF + 2 * CH
WQ_OFF = WZ_OFF + 2 * CH
BLOB_COLS = WQ_OFF + 2 * CH
# rh ships separately as [64, 2*2304] (pair layout, rows 0-63 only)

# scores psum / exp-tile column block per head: two-way row concurrency,
# h0/h2 share psum bank 0 (row block 0), h1/h3 share bank 1 (row block 32)
ECOL = [0, 2, 1, 3]  # head -> 256-col block in psc/et


def _kernel_body(ctx: ExitStack, tc: tile.TileContext, blob_d, rh_d, out_d):
    nc = tc.nc

    consts = ctx.enter_context(tc.tile_pool(name="consts", bufs=1))
    expp = ctx.enter_context(tc.tile_pool(name="expp", bufs=6))
    sump = ctx.enter_context(tc.tile_pool(name="sump", bufs=3))
    outp = ctx.enter_context(tc.tile_pool(name="outp", bufs=9))
    psum_s = ctx.enter_context(tc.tile_pool(name="psum_s", bufs=3, space="PSUM"))
    psum_o = ctx.enter_context(tc.tile_pool(name="psum_o", bufs=2, space="PSUM"))

    # ---- persistent SBUF tensors ----
    blob_sb = consts.tile([128, BLOB_COLS], F32)
    rh_sb = consts.tile([64, 2 * M], F32)
    x_bf = consts.tile([128, 2, M], BF16)
    wv_bf = consts.tile([128, 2, CH], BF16)
    wz_bf = consts.tile([128, 2, CH], BF16)
    wq_bf = consts.tile([128, 2, CH], BF16)
    vh2 = consts.tile([64, 2 * M], BF16)  # [32*(h%2)+c, (h//2)*M + m]
    kz2 = consts.tile([64, 2 * M], BF16)
    qt_sb = consts.tile([128, M], BF16)   # [n, ch] nb-major blocks
    ones_sb = consts.tile([128, DH], BF16)
    zeros_g = consts.tile([128, DH], BF16)  # ghost-matmul weights (all 0)

    # weights land first (small), then x/rh interleaved per chunk with both
    # contraction halves (k=0,1) adjacent, so projection chunk c can start as
    # soon as its ~0.77 MB slice arrives instead of after most of the input.
    nc.sync.dma_start(
        out=blob_sb[:, WV_OFF:BLOB_COLS],
        in_=blob_d.ap()[:, WV_OFF:BLOB_COLS],
    )
    for off, wd in PROJ_CHUNKS:
        for k in range(2):
            nc.sync.dma_start(
                out=blob_sb[:, k * M + off: k * M + off + wd],
                in_=blob_d.ap()[:, k * M + off: k * M + off + wd],
            )
        for pair in range(2):
            nc.sync.dma_start(
                out=rh_sb[:, pair * M + off: pair * M + off + wd],
                in_=rh_d.ap()[:, pair * M + off: pair * M + off + wd],
            )
    nc.vector.memset(ones_sb, 1.0)
    nc.vector.memset(zeros_g, 0.0)

    # bf16 working copies (DVE converts)
    nc.vector.tensor_copy(out=wv_bf, in_=blob_sb[:, WV_OFF:WV_OFF + 2 * CH])
    nc.vector.tensor_copy(out=wz_bf, in_=blob_sb[:, WZ_OFF:WZ_OFF + 2 * CH])
    nc.vector.tensor_copy(out=wq_bf, in_=blob_sb[:, WQ_OFF:WQ_OFF + 2 * CH])

    # zero bias for Exp, produced on ACT so the exp's bias dep is a free
    # same-engine edge rather than an extra cross-engine sync wait
    zero_bias = consts.tile([128, 1], F32)
    nc.scalar.mul(out=zero_bias, in_=blob_sb[:, WV_OFF:WV_OFF + 1], mul=0.0)

    # ---- projections (chunk-major; emission interleaved with attention) ----
    # vh2/kz2: head h -> psum rows 32*(h%2); vh in bank 0, kz in bank 1
    def emit_proj_chunk(off, wd):
        for k in range(2):
            nc.vector.tensor_copy(
                out=x_bf[:, k, off:off + wd],
                in_=blob_sb[:, k * M + off: k * M + off + wd],
            )
        for pair in range(2):
            ps = psum_s.tile([128, 4 * MCH], F32, tag="psc", name="ps")
            for hh in range(2):  # head = 2*pair + hh
                h = 2 * pair + hh
                for k in range(2):
                    nc.tensor.matmul(
                        ps[32 * hh:32 * hh + 32, :wd],
                        lhsT=wv_bf[:, k, 32 * h:32 * h + 32],
                        rhs=x_bf[:, k, off:off + wd],
                        start=(k == 0),
                        stop=(k == 1),
                        tile_position=(0, 32 * hh),
                        skip_group_check=True,
                    )
                    nc.tensor.matmul(
                        ps[32 * hh:32 * hh + 32, 512:512 + wd],
                        lhsT=wz_bf[:, k, 32 * h:32 * h + 32],
                        rhs=x_bf[:, k, off:off + wd],
                        start=(k == 0),
                        stop=(k == 1),
                        tile_position=(0, 32 * hh),
                        skip_group_check=True,
                    )
            nc.vector.tensor_copy(
                out=vh2[:, pair * M + off: pair * M + off + wd], in_=ps[0:64, :wd]
            )
            nc.vector.tensor_add(
                out=kz2[:, pair * M + off: pair * M + off + wd],
                in0=ps[0:64, 512:512 + wd],
                in1=rh_sb[:, pair * M + off: pair * M + off + wd],
            )
        # qhT for the n-blocks covered by this chunk:
        # out[n, ch] = sum_cin x[cin, n] * wT[cin, ch]
        for nb in range(off // 128, (off + wd) // 128):
            ps = psum_s.tile([128, 4 * MCH], F32, tag="psc", name="ps")
            for k in range(2):
                nc.tensor.matmul(
                    ps[:, :CH],
                    lhsT=x_bf[:, k, nb * 128:(nb + 1) * 128],
                    rhs=wq_bf[:, k, :],
                    start=(k == 0),
                    stop=(k == 1),
                )
            nc.vector.tensor_copy(out=qt_sb[:, nb * 128:(nb + 1) * 128], in_=ps[:, :CH])

    # ---- attention ----
    def emit_scores(psc, j, m0):
        # h0 (rows 0-31 -> bank0) || h1 (rows 32-63 -> bank1) concurrent;
        # h2/h3 reuse the same PE rows so they serialize behind h0/h1.
        for h in (0, 1, 2, 3):
            rb = 32 * (h % 2)
            cb = (h // 2) * M
            nc.tensor.matmul(
                psc[:, ECOL[h] * MCH:(ECOL[h] + 1) * MCH],
                lhsT=kz2[rb:rb + 32, cb + j * 128: cb + (j + 1) * 128],
                rhs=vh2[rb:rb + 32, cb + m0: cb + m0 + MCH],
                start=True,
                stop=True,
                tile_position=(rb, 0),
                skip_group_check=True,
            )

    def emit_pv(po, j, et):
        # PV: out_unnorm[32h+d, m] += sum_n qhT[n, 32h+d] * expT_h[n, m]
        for h in range(HPC):
            nc.tensor.matmul(
                po[32 * h:32 * h + 32, 0:MCH],
                lhsT=qt_sb[:, j * 128 + 32 * h: j * 128 + 32 * h + 32],
                rhs=et[:, ECOL[h] * MCH:(ECOL[h] + 1) * MCH],
                start=(j == 0),
                stop=False,
                tile_position=(0, 32 * h),
                skip_group_check=True,
            )

    def emit_sums(po, etp, last):
        # row-sums of an et(j)+et(j+1) pair, replicated over each head slot
        for h in range(HPC):
            nc.tensor.matmul(
                po[32 * h:32 * h + 32, MCH:2 * MCH],
                lhsT=ones_sb,
                rhs=etp[:, ECOL[h] * MCH:(ECOL[h] + 1) * MCH],
                start=False,
                stop=last,
                tile_position=(0, 32 * h),
                skip_group_check=True,
            )

    def emit_pair_sums(po, ets, pj, last):
        etp = sump.tile([128, 4 * MCH], BF16, tag="etp")
        nc.vector.tensor_add(out=etp, in0=ets[pj - 1], in1=ets[pj])
        emit_sums(po, etp, last)

    # Flat software-pipelined loop over g = mc*NB + j. Scores run SLAG steps
    # ahead of exp, which runs one ahead of PV/sums, so the PE queue head
    # never waits on an exp result (head-of-line blocking kept PE idle in
    # bursts, which also pins the HAM clock-gate at 1.2 GHz).
    SLAG = 2  # scores stream leads the exp stream by this many steps
    G = NMCH * NB
    pscs, ets, pos = {}, {}, {}

    def emit_scores_g(g):
        if not (0 <= g < G):
            return
        mc, j = divmod(g, NB)
        pscs[g] = psum_s.tile([128, 4 * MCH], F32, tag="psc", name="psc")
        emit_scores(pscs[g], j, mc * MCH)

    def emit_exp_g(g):
        if not (0 <= g < G):
            return
        _, j = divmod(g, NB)
        psc = pscs[g]
        et = expp.tile([128, 4 * MCH], BF16, tag="et")
        if j in DVE_EXP_J:
            nc.vector.tensor_scalar(
                out=et.bitcast(mybir.dt.int16), in0=psc,
                scalar1=SCHRAUD_A, scalar2=SCHRAUD_B,
                op0=mybir.AluOpType.mult, op1=mybir.AluOpType.add,
            )
        else:
            nc.scalar.activation(
                out=et, in_=psc, func=mybir.ActivationFunctionType.Exp,
                bias=zero_bias, scale=INV_SCALE,
            )
        ets[g] = et

    etps = {}

    def emit_pair_add_g(g):
        # DVE pair-add for sums; emitted BEFORE the next exp on the DVE queue
        # (its inputs are already complete) so the PE's sums matmuls don't
        # stall behind a Schraudolph exp still in the DVE FIFO.
        if not (0 <= g < G) or (g % NB) % 2 != 1:
            return
        etp = sump.tile([128, 4 * MCH], BF16, tag="etp", name="etp")
        nc.vector.tensor_add(out=etp, in0=ets.pop(g - 1), in1=ets[g])
        etps[g] = etp

    def emit_pv_g(g):
        if not (0 <= g < G):
            return
        mc, j = divmod(g, NB)
        if j == 0:
            pos[mc] = psum_o.tile([128, 2 * MCH], F32, tag="po", name="po")
        po = pos[mc]
        emit_pv(po, j, ets[g])
        if j % 2 == 1:
            emit_sums(po, etps.pop(g), last=(j == NB - 1))
        if j % 2 == 1 or j == NB - 1:
            ets.pop(g)
        if j == NB - 1:
            po = pos.pop(mc)
            rc = outp.tile([128, MCH], F32, tag="recip")
            nc.vector.reciprocal_approx_fast(out=rc, in_=po[:, MCH:2 * MCH])
            of = outp.tile([128, MCH], F32, tag="outf")
            nc.vector.tensor_mul(out=of, in0=po[:, 0:MCH], in1=rc)
            m0 = mc * MCH
            nc.sync.dma_start(out=out_d.ap()[:, m0:m0 + MCH], in_=of)

    def emit_ghost(g, idx):
        # Zero-weight matmul on PE rows 64-127 (idle during the scores phase,
        # which only streams rows 0-63) into the already-exp'd psc(g) buffer.
        # Results are x*0 into a dead buffer that scores(g+3) start=True
        # clears; the only purpose is stream activity so the HAM clock gate
        # keeps the PE at 2.4 GHz instead of re-throttling to 1.2.
        if not (NB <= g < G) or g not in pscs:
            return
        mc = g // NB
        m0 = (mc % NMCH) * MCH
        nc.tensor.matmul(
            pscs[g][32 * idx:32 * idx + 32, 0:MCH],
            lhsT=zeros_g[64:128, :],
            rhs=x_bf[64:128, 0, m0:m0 + MCH],
            start=True,
            stop=True,
            tile_position=(64, 32 * idx),
            skip_group_check=True,
        )

    # Interleave projection-chunk emission with the attention pipeline:
    # scores for n-block j only need x/kz chunk j//4, so later chunks project
    # while the first n-blocks' exps already stream on ACT/DVE.
    next_chunk = 0

    def ensure_chunks_for_scores(gs):
        nonlocal next_chunk
        if not (0 <= gs < G):
            need = len(PROJ_CHUNKS)  # pipeline done; flush any stragglers
        else:
            j = gs % NB
            need = min(j // 4, len(PROJ_CHUNKS) - 1) + 1 if gs < NB else len(PROJ_CHUNKS)
        while next_chunk < need:
            emit_proj_chunk(*PROJ_CHUNKS[next_chunk])
            next_chunk += 1

    for g in range(-SLAG, G):
        ensure_chunks_for_scores(g + SLAG)
        emit_scores_g(g + SLAG)
        emit_pair_add_g(g)
        emit_exp_g(g + 1)
        emit_pv_g(g)
        pscs.pop(g, None)

def build_module() -> bass.Bass:
    nc = bacc.Bacc("TRN2", target_bir_lowering=False)
    blob_d = nc.declare_dram_parameter("blob", [128, BLOB_COLS], F32, isOutput=False)
    rh_d = nc.declare_dram_parameter("rh2", [64, 2 * M], F32, isOutput=False)
    out_d = nc.declare_dram_parameter("out", [CH, M], F32, isOutput=True)
    with tile.TileContext(nc) as tc, ExitStack() as ctx:
        _kernel_body(ctx, tc, blob_d, rh_d, out_d)
    nc.compile()
    return nc


def pos_encoding_2d(c, h, w):
    """numpy port of the reference's fixed 2D sinusoidal PE -> (c, h*w)."""
    ch = c // 2
    div = np.float32(10000.0) ** (np.arange(0, ch, 2, dtype=np.float32) / np.float32(ch))
    py = np.arange(h, dtype=np.float32)[None, :] / div[:, None]
    px = np.arange(w, dtype=np.float32)[None, :] / div[:, None]
    pe_y = np.stack([np.sin(py), np.cos(py)], axis=1).reshape(ch, h).astype(np.float32)
    pe_x = np.stack([np.sin(px), np.cos(px)], axis=1).reshape(ch, w).astype(np.float32)
    pe = np.concatenate(
        [
            np.broadcast_to(pe_y[:, :, None], (ch, h, w)),
            np.broadcast_to(pe_x[:, None, :], (ch, h, w)),
        ],
        axis=0,
    )
    return np.ascontiguousarray(pe.reshape(c, h * w), dtype=np.float32)


_CACHE = {}


def _get_nc() -> bass.Bass:
    if "nc" not in _CACHE:
        _CACHE["nc"] = build_module()
    return _CACHE["nc"]


def make_in_maps(x, w_v, w_z, w_q):
    rh_full = pos_encoding_2d(C_OUT, H, W)
    x = np.asarray(x, dtype=np.float32)
    w_v = np.asarray(w_v, dtype=np.float32)
    w_z = np.asarray(w_z, dtype=np.float32)
    w_q = np.asarray(w_q, dtype=np.float32)
    in_maps = []
    for core in range(8):
        b, hh = core // 2, core % 2
        c0 = CH * hh
        blob = np.zeros((128, BLOB_COLS), np.float32)
        # x: blob[p, k*M + j] = x[b, k*128 + p, j]
        xx = x[b].reshape(2, 128, M)
        blob[:, X_OFF:X_OFF + 2 * M] = xx.transpose(1, 0, 2).reshape(128, 2 * M)
        # weights: blob[p, base + k*CH + c] = w[c0 + c, k*128 + p]
        for base, wm in ((WV_OFF, w_v), (WZ_OFF, w_z), (WQ_OFF, w_q)):
            wt = wm[c0:c0 + CH, :].T.reshape(2, 128, CH)  # [k, p, c]
            blob[:, base:base + 2 * CH] = wt.transpose(1, 0, 2).reshape(128, 2 * CH)
        # rh2: [32*(h%2)+c, (h//2)*M + m] = rh[c0 + 32h + c, m]  (rows 0-63)
        rh2 = np.zeros((64, 2 * M), np.float32)
        rh_c = rh_full[c0:c0 + CH, :].reshape(4, DH, M)  # [h, c, m]
        for h in range(4):
            r0 = DH * (h % 2)
            cb = (h // 2) * M
            rh2[r0:r0 + DH, cb:cb + M] = rh_c[h]
        in_maps.append({"blob": blob, "rh2": rh2})
    return in_maps


def assemble_output(results):
    out = np.empty((B, C_OUT, H, W), np.float32)
    for core in range(8):
        b, hh = core // 2, core % 2
        out[b, CH * hh:CH * hh + CH] = results[core]["out"].reshape(CH, H, W)
    return out


def kernel(x, w_v, w_z, w_q, _trace=False):
    nc = _get_nc()
    in_maps = make_in_maps(x, w_v, w_z, w_q)
    res = run_bass_kernel_spmd(nc, in_maps, core_ids=list(range(8)), trace=_trace)
    out = assemble_output(res.results)
    if _trace:
        kernel.last_results = res
    return out

